# revision 40
# baseline (speedup 1.0000x reference)
"""Trainium2 Bass kernel for nn_CrossAttentionModel (8 NeuronCores).

Design (v2):
  * d-parallel encoder: contraction dim D=20480 sharded 8 ways.  Each core
    streams its 2560-row slice of the downsampled activations (all 128
    samples) and of W1/W2, in fp16, packed into 5 large HWDGE DMAs
    (big contiguous descriptors -> near-peak HBM bw).
  * Encoder matmuls in [DE, samples] orientation: psum_aud[e, (b,c,s)],
    psum_vis[e, (b,s)] -- 512 moving rows per k-tile (half of what the
    old 2*DE-wide layout streamed).
  * Cross-core reduce (sum over d-shards, scatter over samples) done with
    remote_dma_broadcast peer SBUF writes instead of a CC ReduceScatter:
    host permutes each core's sample order so that block b holds the
    samples of core (i XOR b); each core sends block r to relative peer
    Delta-tpb=r with a single-dest broadcast (all compile time), then sums
    its 7 received slabs + its own block.  No barrier, no CC stream.
  * Attention phase exploits the replicated-vis structure: the "hi" halves
    of both attention maps are rank-1 outer products, so the whole
    3x3-mixing stage reduces to three K<=3 matmul groups (bd_a, bd_ca,
    bd_wv) plus a tanh-scale trick for the vis/vis quadrant.
  * Residual adds are folded into the final PSUM via K<=3 identity
    matmuls; output leaves as one [128, 96] f32 tile per core.
"""
import sys
sys.path.insert(0, "/opt/trn_rl_repo")

import numpy as np
import concourse.bass as bass
import concourse.mybir as mybir
import concourse.tile as tile
from concourse import bacc
from concourse.bass_utils import run_bass_kernel_spmd

F32 = mybir.dt.float32
F16 = mybir.dt.float16

# ---- problem constants (hardcoded; kernel.py must be self-contained) ----
B, C, H, W = 128, 3, 512, 640
D = 20480            # (H//4) * (W//4)
DE = 128             # encoder dim
DA = 32              # attention dim
NC_ = 8              # cores
DL = D // NC_        # 2560 d-rows per core
NT = DL // 128       # 20 k-tiles per core
SL = B // NC_        # 16 samples per core (post exchange)
SK = SL * DE         # 2048 = (sample, enc-feat) free columns
NQ = 4               # quads of 4 samples
NCHUNK = 5           # input DMA chunks (4 k-tiles each)
DEBUG = False       # dump exchange tiles as extra outputs
KT_PER_CHUNK = NT // NCHUNK
PKW = 384 + 128 + 128 + 128   # per-k-tile packed cols: aT | vT | w1T | w2T

ACT = mybir.ActivationFunctionType


def _np_dt(dt):
    return mybir.dt.np(dt)


def build_bass():
    nc = bacc.Bacc("TRN2", target_bir_lowering=False, debug=False,
                   num_devices=NC_)

    # ---- per-core DRAM parameters ----
    pk = nc.declare_dram_parameter("pk", [128, NT * PKW], F16, isOutput=False)
    bdA = nc.declare_dram_parameter("bdA", [3, 3], F16, isOutput=False)
    rsA = nc.declare_dram_parameter("rsA", [3, 1], F16, isOutput=False)
    csAv = nc.declare_dram_parameter("csAv", [3, 1], F16, isOutput=False)
    alph = nc.declare_dram_parameter("alph", [128, 1], F32, isOutput=False)
    wcaT = nc.declare_dram_parameter("wcaT", [2 * DE, DA], F16, isOutput=False)
    wcvT = nc.declare_dram_parameter("wcvT", [2 * DE, DA], F16, isOutput=False)
    waT = nc.declare_dram_parameter("waT", [3, DA], F16, isOutput=False)
    rsWv = nc.declare_dram_parameter("rsWv", [1, DA], F16, isOutput=False)
    whaT = nc.declare_dram_parameter("whaT", [DA, 3], F16, isOutput=False)
    whvT = nc.declare_dram_parameter("whvT", [DA, 3], F16, isOutput=False)
    i3 = nc.declare_dram_parameter("i3", [3, 3], F16, isOutput=False)
    on13 = nc.declare_dram_parameter("on13", [1, 3], F16, isOutput=False)
    id128 = nc.declare_dram_parameter("id128", [128, 128], F16, isOutput=False)
    out = nc.declare_dram_parameter("out", [128, 6 * SL], F32, isOutput=True)
    if DEBUG:
        dbg_rd = nc.declare_dram_parameter("dbg_rd", [128, 128], F16,
                                           isOutput=True)
        dbg_t = nc.declare_dram_parameter("dbg_t", [128, 128], F16,
                                          isOutput=True)
        dbg_ava = nc.declare_dram_parameter("dbg_ava", [3, SK], F16,
                                            isOutput=True)
        dbg_avv = nc.declare_dram_parameter("dbg_avv", [1, SK], F16,
                                            isOutput=True)
        dbg_bda = nc.declare_dram_parameter("dbg_bda", [3, SK], F16,
                                            isOutput=True)
        dbg_atl = nc.declare_dram_parameter("dbg_atl", [DE, SK], F16,
                                            isOutput=True)
        dbg_hta = nc.declare_dram_parameter("dbg_hta", [DA, SK], F16,
                                            isOutput=True)

    xsem = nc.alloc_semaphore("xchg_sem")
    lsem = nc.alloc_semaphore("xchg_local_sem")
    # scheduler-sim stand-in for xsem: incremented locally by the descgen
    # preps so the single-core scheduling sim can satisfy the reduce gate;
    # the emitted wait is rewritten to xsem after scheduling.
    fksem = nc.alloc_semaphore("fake_xchg_sem")

    # Without any collective in the NEFF the 8 cores launch staggered
    # (~600us apart) and the peer-write flow-sync eats the full skew.
    # A fire-and-forget dummy AllReduce restores the gang launch; nothing
    # waits on its result (the xsem flow-sync absorbs residual skew), so
    # the scheduling sim never blocks on it.
    bar_in = nc.dram_tensor("bar_in", [128, 128], F32)
    bar_out = nc.dram_tensor("bar_out", [128, 128], F32)

    with tile.TileContext(nc) as tc:
        nc.gpsimd.collective_compute(
            "AllReduce", mybir.AluOpType.add,
            replica_groups=[list(range(NC_))],
            ins=[bar_in[:]], outs=[bar_out[:]],
        )
        with (
            tc.tile_pool(name="consts", bufs=1) as cpool,
            tc.tile_pool(name="sb", bufs=1) as sb,
        ):
            # ---------- persistent SBUF tiles ----------
            # input chunks (4 k-tiles each)
            pk_t = [cpool.tile([128, KT_PER_CHUNK * PKW], F16,
                               name=f"pk{cix}", tag=f"pk{cix}")
                    for cix in range(NCHUNK)]
            # exchange tiles
            psend = sb.tile([128, NC_ * 64], F16, name="psend", tag="psend")
            slabs = sb.tile([128, 7 * 64], F16, name="slabs", tag="slabs")
            red = sb.tile([128, 128], F16, name="red", tag="red")

            # ---------- remote-dma descgen (prepare-only; fires later) ----
            for r in range(1, NC_):
                rdests = [None] * NC_
                rdests[r] = (0, r)
                nc.gpsimd.remote_dma_broadcast(
                    slabs[:, (r - 1) * 64:r * 64],
                    psend[:, r * 64:(r + 1) * 64],
                    remote_sem=xsem, local_sem=lsem, rdests=rdests)

            # ---------- const loads (scalar engine HWDGE) ----------
            bdA_t = cpool.tile([3, 3], F16)
            nc.scalar.dma_start(bdA_t[:], bdA[:])
            rsA_t = cpool.tile([3, 1], F16)
            nc.scalar.dma_start(rsA_t[:], rsA[:])
            csAv_t = cpool.tile([3, 1], F16)
            nc.scalar.dma_start(csAv_t[:], csAv[:])
            alph_t = cpool.tile([128, 1], F32)
            nc.scalar.dma_start(alph_t[:], alph[:])
            wca_lo = cpool.tile([DE, DA], F16)
            nc.scalar.dma_start(wca_lo[:], wcaT[0:DE, :])
            wca_hi = cpool.tile([DE, DA], F16)
            nc.scalar.dma_start(wca_hi[:], wcaT[DE:2 * DE, :])
            wcv_lo = cpool.tile([DE, DA], F16)
            nc.scalar.dma_start(wcv_lo[:], wcvT[0:DE, :])
            wcv_hi = cpool.tile([DE, DA], F16)
            nc.scalar.dma_start(wcv_hi[:], wcvT[DE:2 * DE, :])
            waT_t = cpool.tile([3, DA], F16)
            nc.scalar.dma_start(waT_t[:], waT[:])
            rsWv_t = cpool.tile([1, DA], F16)
            nc.scalar.dma_start(rsWv_t[:], rsWv[:])
            wha_t = cpool.tile([DA, 3], F16)
            nc.scalar.dma_start(wha_t[:], whaT[:])
            whv_t = cpool.tile([DA, 3], F16)
            nc.scalar.dma_start(whv_t[:], whvT[:])
            i3_t = cpool.tile([3, 3], F16)
            nc.scalar.dma_start(i3_t[:], i3[:])
            on13_t = cpool.tile([1, 3], F16)
            nc.scalar.dma_start(on13_t[:], on13[:])
            id_t = cpool.tile([128, 128], F16)
            nc.scalar.dma_start(id_t[:], id128[:])

            # ---------- input chunk loads (sync engine HWDGE) ----------
            for cix in range(NCHUNK):
                c0 = cix * KT_PER_CHUNK * PKW
                nc.sync.dma_start(
                    pk_t[cix][:], pk[:, c0:c0 + KT_PER_CHUNK * PKW])

            # ---------- phase 1: encoder ----------
            with tc.tile_pool(name="enc_ps", bufs=1, space="PSUM") as eps:
                ps_aud = eps.tile([128, 384], F32, name="ps_aud")
                ps_vis = eps.tile([128, 128], F32, name="ps_vis")
                for t in range(NT):
                    cix, tloc = divmod(t, KT_PER_CHUNK)
                    o = tloc * PKW
                    src = pk_t[cix]
                    first, last = t == 0, t == NT - 1
                    nc.tensor.matmul(ps_aud[:], src[:, o + 512:o + 640],
                                     src[:, o:o + 384],
                                     start=first, stop=last)
                    nc.tensor.matmul(ps_vis[:], src[:, o + 640:o + 768],
                                     src[:, o + 384:o + 512],
                                     start=first, stop=last)

                # evict partials (f32 -> f16) into block-major send tile.
                # On the scalar engine, NOT DVE: DVE opens with the
                # (rewritten) xsem wait and must never gate our own sends.
                ev1 = nc.scalar.copy(
                    psend[:].rearrange("p (b w) -> p b w", w=64)[:, :, 0:48],
                    ps_aud[:].rearrange("p (b w) -> p b w", w=48))
                ev2 = nc.scalar.copy(
                    psend[:].rearrange("p (b w) -> p b w", w=64)[:, :, 48:64],
                    ps_vis[:].rearrange("p (b w) -> p b w", w=16))

            # ---------- exchange: fire the 7 peer writes ----------
            # signals_writable=psend makes the trigger a tile-visible
            # writer of psend, ordering it AFTER the evict copies (the
            # preps' deferred reads alone leave the trigger unordered
            # against later writers of its source).
            trig = nc.gpsimd.trigger_dma(count=None,
                                         signals_writable=[psend[:]])
            # sim-only stand-in for the 14 remote arrivals (see fksem above)
            nc.gpsimd.sem_inc(fksem, 14)

            # ---------- reduce: own block + 7 slabs ----------
            # Standalone DVE wait on fksem (sim-satisfiable: preps bump it
            # to 14 locally); rewritten to xsem>=14 after scheduling so the
            # hardware actually gates on the 7 peer-write arrivals.
            fkw = nc.vector.wait_ge(fksem, 14)
            gate = nc.vector.tensor_copy(red[:, 0:64], psend[:, 0:64])
            adds = []
            for r in range(1, NC_):
                adds.append(nc.vector.tensor_add(
                    red[:, 0:64], red[:, 0:64],
                    slabs[:, (r - 1) * 64:r * 64]))

            if DEBUG:
                nc.sync.dma_start(dbg_rd[:], red[:])

            # ---------- transpose to per-channel layout ----------
            # red[k, c*16+s] (aud c<3; vis at 48+s) -> av_a[c, (s,k)],
            # av_v[0, (s,k)] via per-sample PE transposes (stride-16
            # column APs; psum evicts are partition-preserving).
            av_a = sb.tile([3, SK], F16, name="av_a", tag="av_a")
            av_v = sb.tile([1, SK], F16, name="av_v", tag="av_v")
            red_cs = red[:].rearrange("k (c s) -> k c s", s=SL)
            with tc.tile_pool(name="tr_ps", bufs=2, space="PSUM") as trp:
                for s in range(SL):
                    sl_ = slice(s * DE, (s + 1) * DE)
                    pa_t = trp.tile([3, DE], F16, tag="tr_a")
                    nc.tensor.transpose(pa_t[:], red_cs[:, 0:3, s:s + 1],
                                        id_t[:])
                    nc.scalar.copy(av_a[:, sl_], pa_t[:])
                    pv_t = trp.tile([1, DE], F16, tag="tr_v")
                    nc.tensor.transpose(pv_t[:], red_cs[:, 3:4, s:s + 1],
                                        id_t[:])
                    nc.vector.tensor_copy(av_v[:, sl_], pv_t[:])
            if DEBUG:
                avacp = sb.tile([3, SK], F16, name="avacp", tag="avacp")
                nc.vector.tensor_copy(avacp[:], av_a[:])
                nc.sync.dma_start(dbg_ava[:], avacp[:])
                avvcp = sb.tile([1, SK], F16, name="avvcp", tag="avvcp")
                nc.vector.tensor_copy(avvcp[:], av_v[:])
                nc.sync.dma_start(dbg_avv[:], avvcp[:])

            # ---------- phase 2: attention (16 samples, on-chip) ----------
            bd_a = sb.tile([3, SK], F16, name="bd_a", tag="bd_a")
            bd_ca = sb.tile([1, SK], F16, name="bd_ca", tag="bd_ca")
            bd_wv = sb.tile([1, SK], F16, name="bd_wv", tag="bd_wv")
            att = {
                (br, half): sb.tile([DE, SK], F16, name=f"att_{br}_{half}",
                                    tag=f"att_{br}_{half}")
                for br in ("a", "v") for half in ("lo", "hi")
            }
            ht_a = sb.tile([DA, SK], F16, name="ht_a", tag="ht_a")
            ht_v = sb.tile([DA, SK], F16, name="ht_v", tag="ht_v")

            with (
                tc.tile_pool(name="bd_ps", bufs=1, space="PSUM") as bps,
                tc.tile_pool(name="att_ps", bufs=2, space="PSUM") as aps,
                tc.tile_pool(name="h_ps", bufs=1, space="PSUM") as hps,
                tc.tile_pool(name="o_ps", bufs=1, space="PSUM") as ops_,
            ):
                # ---- bd stage: aud' rows, csum_a row, w_v row ----
                # evict-casts split across DVE and Act so the bd->att
                # chain isn't serialized on one engine
                for q in range(NQ):
                    ck = slice(q * 512, (q + 1) * 512)
                    pa = bps.tile([3, 512], F32, tag="pbd_a")
                    nc.tensor.matmul(pa[:], bdA_t[:], av_a[:, ck],
                                     start=True, stop=True)
                    nc.vector.tensor_copy(bd_a[:, ck], pa[:])
                    pc = bps.tile([1, 512], F32, tag="pbd_c")
                    nc.tensor.matmul(pc[:], rsA_t[:], av_a[:, ck],
                                     start=True, stop=True)
                    nc.scalar.copy(bd_ca[:, ck], pc[:])
                    pw = bps.tile([1, 512], F32, tag="pbd_w")
                    nc.tensor.matmul(pw[:], csAv_t[:], av_a[:, ck],
                                     start=True, stop=True)
                    nc.vector.tensor_copy(bd_wv[:, ck], pw[:])

                # ---- attention maps ----
                # att_a_lo[m,k] = sum_j aud[j,m] * aud'[j,k]
                # att_a_hi[m,k] = vis[m] * csum_a[k]
                # att_v_lo[m,k] = w_v[m] * vis[k]
                # att_v_hi[m,k] = alpha * vis[m] * vis[k]   (alpha in tanh)
                aspec = [("a", "lo", av_a, bd_a), ("a", "hi", av_v, bd_ca),
                         ("v", "lo", bd_wv, av_v), ("v", "hi", av_v, av_v)]
                for q in range(NQ):
                    for br, half, lhs_t, rhs_t in aspec:
                        pt = aps.tile([DE, 512], F32, tag="att_ps")
                        for j in range(4):
                            s = q * 4 + j
                            sl_ = slice(s * DE, (s + 1) * DE)
                            nc.tensor.matmul(pt[:, j * DE:(j + 1) * DE],
                                             lhs_t[:, sl_], rhs_t[:, sl_],
                                             start=True, stop=True)
                        dst = att[(br, half)][:, q * 512:(q + 1) * 512]
                        if br == "v" and half == "hi":
                            nc.scalar.activation(dst, pt[:], ACT.Tanh,
                                                 scale=alph_t[:])
                        else:
                            nc.scalar.activation(dst, pt[:], ACT.Tanh)

                if DEBUG:
                    nc.sync.dma_start(dbg_bda[:], bd_a[:])
                    nc.sync.dma_start(dbg_atl[:], att[("a", "lo")][:])

                # ---- H = relu(att @ WcT + enc-term) ----
                for q in range(NQ):
                    ck = slice(q * 512, (q + 1) * 512)
                    ph_a = hps.tile([DA, 512], F32, tag="ph_a")
                    nc.tensor.matmul(ph_a[:], waT_t[:], av_a[:, ck],
                                     start=True, stop=False)
                    nc.tensor.matmul(ph_a[:], wca_lo[:], att[("a", "lo")][:, ck],
                                     start=False, stop=False)
                    nc.tensor.matmul(ph_a[:], wca_hi[:], att[("a", "hi")][:, ck],
                                     start=False, stop=True)
                    nc.scalar.activation(ht_a[:, ck], ph_a[:], ACT.Relu)
                    ph_v = hps.tile([DA, 512], F32, tag="ph_v")
                    nc.tensor.matmul(ph_v[:], rsWv_t[:], av_v[:, ck],
                                     start=True, stop=False)
                    nc.tensor.matmul(ph_v[:], wcv_lo[:], att[("v", "lo")][:, ck],
                                     start=False, stop=False)
                    nc.tensor.matmul(ph_v[:], wcv_hi[:], att[("v", "hi")][:, ck],
                                     start=False, stop=True)
                    nc.scalar.activation(ht_v[:, ck], ph_v[:], ACT.Relu)

                if DEBUG:
                    nc.sync.dma_start(dbg_hta[:], ht_a[:])

                # ---- out[k, (s, c|3+c)] = Wh @ H (+ residual at evict) ----
                po = ops_.tile([128, 6 * SL], F32, name="po", tag="po")
                for s in range(SL):
                    sl_ = slice(s * DE, (s + 1) * DE)
                    nc.tensor.matmul(po[:, s * 6:s * 6 + 3],
                                     ht_a[:, sl_], wha_t[:],
                                     start=True, stop=True)
                    nc.tensor.matmul(po[:, s * 6 + 3:s * 6 + 6],
                                     ht_v[:, sl_], whv_t[:],
                                     start=True, stop=True)

                # residual add straight from `red` (enc^T layout) -- saves
                # 32 LDWEIGHTS-heavy identity matmuls at the tail
                out_sb = sb.tile([128, 6 * SL], F32, name="out_sb",
                                 tag="out_sb")
                po_v = po[:].rearrange("k (s c) -> k s c", c=6)
                ob_v = out_sb[:].rearrange("k (s c) -> k s c", c=6)
                red_a = red[:, 0:48].rearrange("k (c s) -> k s c", s=SL)
                nc.vector.tensor_add(ob_v[:, :, 0:3], po_v[:, :, 0:3], red_a)
                red_v = red[:, 48:64].rearrange("k (s c) -> k s c", c=1)
                for c in range(3):
                    nc.vector.tensor_add(ob_v[:, :, 3 + c:4 + c],
                                         po_v[:, :, 3 + c:4 + c], red_v)
                nc.sync.dma_start(out[:], out_sb[:])

    # ---- post-schedule surgery + safety checks ----
    # 1. rewrite the standalone DVE wait from fksem to xsem (peer arrivals)
    waits = fkw.ins.sync_info.on_wait
    assert len(waits) == 1 and waits[0].id == fksem.num, waits
    waits[0].id = xsem.num
    assert fkw.ins.sync_info.on_wait[0].id == xsem.num

    all_insts = [i for bb in nc.m.functions[0].blocks for i in bb.instructions]
    dve = [i for i in all_insts if str(i.engine) == "EngineType.DVE"]
    dve_pos = {i.name: p for p, i in enumerate(dve)}
    wpos = dve_pos[fkw.ins.name]

    # 2. the wait must precede the gate copy and every slab-reading add in
    #    the DVE stream (engines execute their stream in order).
    assert wpos < dve_pos[gate.ins.name], "gate copy before xsem wait"
    for a in adds:
        assert wpos < dve_pos[a.ins.name], "slab add before xsem wait"

    # 3. the send path (trigger_dma and its transitive wait-upstream) must
    #    not depend on any DVE instruction at/after the xsem wait, or all
    #    cores would deadlock waiting for sends that never fire.
    # A wait (sem, v) only depends on the PREFIX of the sem's updaters (in
    # stream order) whose cumulative increments reach v.
    post_wait_dve = {i.name for i in dve[wpos:]}
    gpos = {i.name: p for p, i in enumerate(all_insts)}
    sem_updaters = {}
    for i in all_insts:
        si = i.sync_info
        if si is None:
            continue
        for u in si.on_update:
            v = u.update_value if u.update_value else 1
            if v > 0:
                sem_updaters.setdefault(u.id, []).append((gpos[i.name], i, v))
    for ups in sem_updaters.values():
        ups.sort(key=lambda t: t[0])
    seen = set()
    stack = [trig.ins]
    while stack:
        i = stack.pop()
        if i.name in seen:
            continue
        seen.add(i.name)
        assert i.name not in post_wait_dve, \
            f"send path depends on post-wait DVE inst {i.name}"
        si = i.sync_info
        if si is None:
            continue
        for wt in si.on_wait:
            need = wt.wait_value or 0
            acc = 0
            for _, up, v in sem_updaters.get(wt.id, []):
                if acc >= need:
                    break
                acc += v
                if up.name not in seen:
                    stack.append(up)

    nc.compile()
    return nc


_NC_CACHE = None


def _get_nc():
    global _NC_CACHE
    if _NC_CACHE is None:
        _NC_CACHE = build_bass()
    return _NC_CACHE


def _prep_inputs(f1_norm, f2_norm, W1, b1, W2, b2, Aa, Av, Wa, Wv,
                 Wca, Wcv, Wha, Whv):
    f16 = _np_dt(F16)
    f1_norm = np.asarray(f1_norm, dtype=np.float32)
    f2_norm = np.asarray(f2_norm, dtype=np.float32)
    Aa = np.asarray(Aa, dtype=np.float32)
    Av = np.asarray(Av, dtype=np.float32)

    a_ds = f1_norm[:, :, ::4, ::4].reshape(B, 3, D)       # (B, 3, D)
    v_ds = f2_norm[:, ::4, ::4].reshape(B, D)             # (B, D)
    w1T = np.ascontiguousarray(np.asarray(W1).T).astype(f16)   # (D, 128)
    w2T = np.ascontiguousarray(np.asarray(W2).T).astype(f16)

    scale = 1.0 / 16.0
    consts = {
        "bdA": (Aa * scale).astype(f16),
        "rsA": (Aa.sum(axis=1, keepdims=True) * scale).astype(f16),
        "csAv": (Av.sum(axis=0)[:, None] * scale).astype(f16),
        "alph": np.full((128, 1), Av.sum() * scale, np.float32),
        "wcaT": np.ascontiguousarray(np.asarray(Wca).T).astype(f16),
        "wcvT": np.ascontiguousarray(np.asarray(Wcv).T).astype(f16),
        "waT": np.ascontiguousarray(np.asarray(Wa).T).astype(f16),
        "rsWv": np.asarray(Wv).sum(axis=1)[None, :].astype(f16),
        "whaT": np.ascontiguousarray(np.asarray(Wha).T).astype(f16),
        "whvT": np.ascontiguousarray(np.asarray(Whv).T).astype(f16),
        "i3": np.eye(3, dtype=f16),
        "on13": np.ones((1, 3), f16),
        "id128": np.eye(128, dtype=f16),
    }

    in_maps = []
    # Cross-die peer writes (Delta-tpb bit 2 set) land on core dest^2 --
    # the D2D hop flips tpb bit 1 (measured, consistent across all 64
    # core/slot pairs).  Compensate in the host block permutation: block b
    # holds the samples of core (i ^ g(b)) with g flipping bit 1 for the
    # cross-die blocks; the misrouting then delivers every block to the
    # core whose samples it carries.
    g = [b ^ 2 if b & 4 else b for b in range(NC_)]
    for i in range(NC_):
        dsl = slice(i * DL, (i + 1) * DL)
        perm = np.concatenate(
            [np.arange((i ^ g[b]) * SL, ((i ^ g[b]) + 1) * SL)
             for b in range(NC_)])
        # aT: [d, (b, c, s)]
        ab = a_ds[perm].reshape(NC_, SL, 3, D)[..., dsl]   # (8, 16, 3, 2560)
        aT = np.ascontiguousarray(ab.transpose(3, 0, 2, 1)
                                  ).reshape(DL, NC_ * 48).astype(f16)
        vb = v_ds[perm].reshape(NC_, SL, D)[..., dsl]      # (8, 16, 2560)
        vT = np.ascontiguousarray(vb.transpose(2, 0, 1)
                                  ).reshape(DL, NC_ * SL).astype(f16)
        pk = np.concatenate([
            aT.reshape(NT, 128, 384), vT.reshape(NT, 128, 128),
            w1T[dsl].reshape(NT, 128, 128), w2T[dsl].reshape(NT, 128, 128),
        ], axis=2).transpose(1, 0, 2).reshape(128, NT * PKW)
        m = {"pk": np.ascontiguousarray(pk)}
        m.update(consts)
        in_maps.append(m)
    return in_maps


def _unshard(res):
    outs = []
    for i in range(NC_):
        arr = res.results[i]["out"].reshape(128, SL, 6)    # [k, s, 6]
        aud = arr[:, :, 0:3].transpose(1, 2, 0)            # (16, 3, 128)
        vis = arr[:, :, 3:6].transpose(1, 2, 0)
        outs.append(np.concatenate([aud, vis], axis=2))    # (16, 3, 256)
    return np.concatenate(outs, axis=0).astype(np.float32, copy=False)


def _run(inputs, trace=False):
    nc = _get_nc()
    in_maps = _prep_inputs(**inputs)
    res = run_bass_kernel_spmd(nc, in_maps, list(range(NC_)), trace=trace)
    return _unshard(res), res


def kernel(**inputs):
    out, _ = _run(inputs, trace=False)
    return out
